# revision 39
# baseline (speedup 1.0000x reference)
"""Trainium2 Bass kernel for nn_CPProfileSurrogate (CP-factored profile surrogate).

Data-parallel over batch across 8 NeuronCores. Each core handles B/8 = 4096
rows:
    mu_h  = x @ mu_w.T + mu_b                       (PE, contraction 256)
    std_h = softplus(x @ std_w.T + std_b)           (ACT)
    h[s]  = mu_h + std_h * eps[s]                   (DVE)
    z     = h @ C ; t = A[g] * z                    (DVE, tiny contractions)
    logits= t @ B.T (+ b[g])                        (PE, contraction R=4)
    out   = softmax(logits)                         (DVE max, ACT exp, scale)

b[g] is handled via a gathered exp(b) factor only when b != 0 (the common
case b == 0 skips it): softmax(l + bg) == exp(l - m) * exp(bg) / sum.
"""

import numpy as np

import concourse.bass as bass
import concourse.tile as tile
import concourse.mybir as mybir
from concourse import bass_utils

NCORES = 8
B_SZ = 32768
S = 4
D_IN = 256
LAT = 8
NBINS = 300
K = 1323
R = 4

P = 128
BS = B_SZ // NCORES          # rows per core
NT = BS // P                 # batch tiles per core
NSLOT = S + 1                # 4 mc samples + 1 mean path
F32 = mybir.dt.float32
F32R = mybir.dt.float32r    # 4-byte fp32, 4-xbus streaming: 1 cyc/row on PE
I32 = mybir.dt.int32

# Hull-max columns appended to the logits matmul: max_k t.B_k equals the max
# over convex-hull vertices of B's rows, so the softmax max costs V extra
# matmul columns + a short reduce instead of a 1323-wide reduce. VMAX keeps
# the padded logits tile within 3 PSUM banks (1536 fp32).
VMAX = 213


def _split_multiwait(nc, max_waits=1):
    """The walrus build in this container rejects >1 sync-wait per
    instruction; hoist extra waits onto preceding single-wait NoOps."""
    for f in nc.m.functions:
        for bb in f.blocks:
            insts = bb.instructions
            new = []
            changed = False
            for inst in insts:
                si = inst.sync_info
                waits = list(si.on_wait) if si is not None else []
                if len(waits) > max_waits:
                    extra, keep = waits[:-max_waits], waits[-max_waits:]
                    for k, w in enumerate(extra):
                        new.append(mybir.InstNoOp(
                            name=f"{inst.name}_sw{k}",
                            engine=inst.engine,
                            bass_nofuse=True,
                            sync_info=mybir.SyncInfo(on_wait=[w], on_update=[]),
                        ))
                    inst.sync_info = mybir.SyncInfo(
                        on_wait=keep, on_update=list(si.on_update))
                    changed = True
                new.append(inst)
            if changed:
                bb.instructions = new


_NC_CACHE = {}


def build_nc(use_b: bool, V: int = 0):
    """V = number of appended hull-max columns (0 -> plain full reduce_max)."""
    key = (use_b, V)
    if key in _NC_CACHE:
        return _NC_CACHE[key]
    # fp32r matmuls need 4-aligned moving widths: zero-pad the table
    KX = K + V + (4 - ((K + V - 1024) % 4)) % 4
    nc = bass.Bass("TRN2", target_bir_lowering=False, debug=False)

    x_d = nc.dram_tensor("x", [BS, D_IN], F32, kind="ExternalInput")
    eps_d = nc.dram_tensor("epsr", [BS, S * LAT], F32, kind="ExternalInput")
    lab_d = nc.dram_tensor("labs", [P, NT], I32, kind="ExternalInput")
    w2_d = nc.dram_tensor("w2", [D_IN, 16], F32, kind="ExternalInput")
    bias_d = nc.dram_tensor("biasrow", [1, 16], F32, kind="ExternalInput")
    cb_d = nc.dram_tensor("cb", [P, R * LAT], F32, kind="ExternalInput")
    id_d = nc.dram_tensor("ident", [P, P], F32, kind="ExternalInput")
    bt_d = nc.dram_tensor("btab", [R, KX], F32R, kind="ExternalInput")
    a_d = nc.dram_tensor("atab", [NBINS, R], F32, kind="ExternalInput")
    if use_b:
        eb_d = nc.dram_tensor("ebtab", [NBINS, K], F32, kind="ExternalInput")

    zp_d = nc.dram_tensor("zp", [S, BS, K], F32, kind="ExternalOutput")
    mp_d = nc.dram_tensor("mp", [BS, K], F32, kind="ExternalOutput")
    mu_d = nc.dram_tensor("muh", [BS, LAT], F32, kind="ExternalOutput")
    sd_d = nc.dram_tensor("sdh", [BS, LAT], F32, kind="ExternalOutput")

    AF = mybir.ActivationFunctionType
    AX = mybir.AxisListType
    OP = mybir.AluOpType

    with tile.TileContext(nc) as tc:
        with (
            tc.tile_pool(name="const", bufs=1) as cp,
            tc.tile_pool(name="work", bufs=4) as wp,
            tc.tile_pool(name="osb", bufs=8) as op_,
            tc.tile_pool(name="psL", bufs=2, space="PSUM") as psL,
            tc.tile_pool(name="psS", bufs=2, space="PSUM") as psS,
        ):
            ident = cp.tile([P, P], F32)
            nc.scalar.dma_start(ident[:], id_d.ap()[:])
            ws = cp.tile([P, 2, 16], F32)
            nc.scalar.dma_start(ws[:], w2_d.ap().rearrange("(c p) n -> p c n", p=P))
            biasr = cp.tile([1, 16], F32)
            nc.scalar.dma_start(biasr[:], bias_d.ap()[:])
            ones1 = cp.tile([1, P], F32)
            nc.gpsimd.memset(ones1[:], 1.0)
            cb = cp.tile([P, R * LAT], F32)
            nc.scalar.dma_start(cb[:], cb_d.ap()[:])
            bt = cp.tile([R, KX], F32R)
            nc.scalar.dma_start(bt[:], bt_d.ap()[:])
            labs = cp.tile([P, NT], I32)
            nc.scalar.dma_start(labs[:], lab_d.ap()[:])

            def frontend(i):
                """Loads + linear + h/z/t + per-slot transposes for tile i.
                Returns what the softmax backend needs."""
                r0, r1 = i * P, (i + 1) * P
                xt = wp.tile([P, D_IN], F32, tag="xt")
                nc.scalar.dma_start(xt[:], x_d.ap()[r0:r1, :])
                ep = wp.tile([P, S * LAT], F32, tag="ep")
                nc.scalar.dma_start(ep[:], eps_d.ap()[r0:r1, :])
                an = wp.tile([P, R], F32, tag="an")
                nc.gpsimd.indirect_dma_start(
                    out=an[:], out_offset=None,
                    in_=a_d.ap()[:],
                    in_offset=bass.IndirectOffsetOnAxis(ap=labs[:, i:i + 1], axis=0),
                )
                ebn = None
                if use_b:
                    ebn = wp.tile([P, K], F32, tag="ebn")
                    nc.gpsimd.indirect_dma_start(
                        out=ebn[:], out_offset=None,
                        in_=eb_d.ap()[:],
                        in_offset=bass.IndirectOffsetOnAxis(ap=labs[:, i:i + 1], axis=0),
                    )

                # x.T via PE so the linear layer can contract over D_IN
                xtp = psS.tile([P, D_IN], F32, tag="pss")
                nc.tensor.transpose(xtp[:, 0:P], xt[:, 0:P], ident[:])
                nc.tensor.transpose(xtp[:, P:D_IN], xt[:, P:D_IN], ident[:])
                xts = wp.tile([P, D_IN], F32, tag="xts")
                nc.vector.tensor_copy(xts[:], xtp[:])

                # [mu_h | std_pre] = x @ [mu_w|std_w].T + [mu_b|std_b]
                lin = psS.tile([P, 16], F32, tag="pss")
                nc.tensor.matmul(out=lin[:], lhsT=xts[:, 0:P], rhs=ws[:, 0, :],
                                 start=True, stop=False)
                nc.tensor.matmul(out=lin[:], lhsT=xts[:, P:D_IN], rhs=ws[:, 1, :],
                                 start=False, stop=False)
                nc.tensor.matmul(out=lin[:], lhsT=ones1[:], rhs=biasr[:],
                                 start=False, stop=True)

                # h_all: slots 0..3 = mu+std*eps[s], slot 4 = mu (mean path)
                ha = wp.tile([P, NSLOT * LAT], F32, tag="ha")
                nc.vector.tensor_copy(ha[:, S * LAT:], lin[:, 0:LAT])
                # softplus(x) = ln(exp(x) + 1); this ACT table build has no
                # native softplus, but exp and ln share one table set
                sd = wp.tile([P, LAT], F32, tag="sd")
                nc.scalar.activation(sd[:], lin[:, LAT:16], AF.Exp)
                nc.scalar.activation(sd[:], sd[:], AF.Ln, bias=1.0)
                # small stores ride the idle Pool SWDGE queue so they can't
                # head-block the SP FIFO that carries the big zp stores
                nc.gpsimd.dma_start(mu_d.ap()[r0:r1, :], ha[:, S * LAT:])
                nc.gpsimd.dma_start(sd_d.ap()[r0:r1, :], sd[:])

                ha4 = ha[:, 0:S * LAT].rearrange("p (s l) -> p s l", s=S)
                sdb = sd[:].rearrange("p (o l) -> p o l", o=1).to_broadcast([P, S, LAT])
                mub = ha[:, S * LAT:].rearrange("p (o l) -> p o l", o=1).to_broadcast([P, S, LAT])
                ep4 = ep[:].rearrange("p (s l) -> p s l", s=S)
                nc.vector.tensor_tensor(out=ha4, in0=ep4, in1=sdb, op=OP.mult)
                nc.vector.tensor_tensor(out=ha4, in0=ha4, in1=mub, op=OP.add)

                # z[p, slot, r] = sum_l h[p, slot, l] * C[l, r]
                prod = wp.tile([P, NSLOT, R, LAT], F32, tag="prod")
                hab = ha[:].rearrange("p (s o l) -> p s o l", s=NSLOT, o=1) \
                           .to_broadcast([P, NSLOT, R, LAT])
                cbb = cb[:].rearrange("p (o r l) -> p o r l", o=1, r=R) \
                           .to_broadcast([P, NSLOT, R, LAT])
                nc.vector.tensor_tensor(out=prod[:], in0=hab, in1=cbb, op=OP.mult)
                zt = wp.tile([P, NSLOT, R], F32, tag="zt")
                nc.vector.reduce_sum(out=zt[:], in_=prod[:], axis=AX.X)
                anb = an[:].rearrange("p (o r) -> p o r", o=1).to_broadcast([P, NSLOT, R])
                nc.vector.tensor_tensor(out=zt[:], in0=zt[:], in1=anb, op=OP.mult)

                # transpose each slot's t -> (R, P) so PE can contract over R
                # (PE/PSUM partition bases must be 32-aligned, so one slot at
                # a time, each landing at partition 0)
                tts = []
                for s in range(NSLOT):
                    ttp_s = psS.tile([R, P], F32, tag="pss")
                    nc.tensor.transpose(ttp_s[:], zt[:, s, :], ident[:])
                    t_s = wp.tile([R, P], F32R, tag=f"tts{s}")
                    nc.vector.tensor_copy(t_s[:], ttp_s[:])
                    tts.append(t_s)
                return tts, ebn

            def backend(i, st, slots):
                """Logits + softmax + stores for tile i, given softmax slots."""
                tts, ebn = st
                r0, r1 = i * P, (i + 1) * P
                kch = [(0, 512), (512, 1024), (1024, KX)]
                for s in slots:
                    L = psL.tile([P, KX], F32, tag="L")
                    for (c0, c1) in kch:
                        nc.tensor.matmul(out=L[:, c0:c1],
                                         lhsT=tts[s][:],
                                         rhs=bt[:, c0:c1],
                                         start=True, stop=True)
                    nmax = wp.tile([P, 1], F32, tag="nmax")
                    if V:
                        # appended hull columns carry max_k(t . B_k) exactly
                        nc.vector.reduce_max(out=nmax[:], in_=L[:, K:K + V], axis=AX.X)
                    else:
                        nc.vector.reduce_max(out=nmax[:], in_=L[:, 0:K], axis=AX.X)
                    nc.vector.tensor_scalar_mul(nmax[:], nmax[:], -1.0)
                    osb = op_.tile([P, K], F32, tag="osb")
                    ssum = wp.tile([P, 1], F32, tag="ssum")
                    if use_b:
                        tmp = wp.tile([P, K], F32, tag="tmpe")
                        nc.scalar.activation(tmp[:], L[:, 0:K], AF.Exp, bias=nmax[:])
                        nc.vector.tensor_tensor(out=osb[:], in0=tmp[:], in1=ebn[:],
                                                op=OP.mult)
                        nc.vector.reduce_sum(out=ssum[:], in_=osb[:], axis=AX.X)
                    else:
                        nc.scalar.activation(osb[:], L[:, 0:K], AF.Exp, bias=nmax[:],
                                             accum_out=ssum[:])
                    rinv = wp.tile([P, 1], F32, tag="rinv")
                    nc.vector.reciprocal(rinv[:], ssum[:])
                    nc.vector.tensor_scalar_mul(osb[:], osb[:], rinv[:])
                    if s < S:
                        nc.sync.dma_start(zp_d.ap()[s, r0:r1, :], osb[:])
                    else:
                        nc.sync.dma_start(mp_d.ap()[r0:r1, :], osb[:])

            # 1-deep software pipeline: emit tile i+1's frontend before tile
            # i's softmax loop so the in-order PE always has store-feeding
            # matmul work queued behind the (short) next-tile prep.
            st = frontend(0)
            for i in range(NT):
                nxt = frontend(i + 1) if i + 1 < NT else None
                backend(i, st, range(NSLOT))
                st = nxt

    _split_multiwait(nc)
    _NC_CACHE[key] = nc
    return nc


def kernel(x, eps, A, B, C, b, mu_w, mu_b, std_w, std_b, group_labels):
    x = np.ascontiguousarray(np.asarray(x, np.float32))
    eps = np.asarray(eps, np.float32)
    A = np.ascontiguousarray(np.asarray(A, np.float32))
    Bm = np.ascontiguousarray(np.asarray(B, np.float32))
    C = np.asarray(C, np.float32)
    b = np.asarray(b, np.float32)
    mu_w = np.asarray(mu_w, np.float32)
    mu_b = np.asarray(mu_b, np.float32)
    std_w = np.asarray(std_w, np.float32)
    std_b = np.asarray(std_b, np.float32)
    labels = np.asarray(group_labels)

    use_b = bool(np.any(b))

    # convex-hull vertices of B's rows: the row-max of t @ B.T is attained at
    # a hull vertex, so V extra matmul columns give the softmax max exactly
    try:
        from scipy.spatial import ConvexHull
        verts = list(np.sort(ConvexHull(Bm.astype(np.float64)).vertices))
        # fp32r matmuls need 4-aligned moving widths; pad the last chunk
        while (K - 1024 + len(verts)) % 4:
            verts.append(verts[0])
        verts = np.asarray(verts)
        V = len(verts) if len(verts) <= VMAX else 0
    except Exception:
        verts, V = None, 0

    nc = build_nc(use_b, V)

    w2 = np.ascontiguousarray(np.concatenate([mu_w, std_w], 0).T)      # (256,16)
    biasrow = np.concatenate([mu_b, std_b])[None, :].copy()            # (1,16)
    cb = np.tile(np.ascontiguousarray(C.T).reshape(1, R * LAT), (P, 1))  # (128,32)
    bt = np.ascontiguousarray(Bm.T)                                    # (4,1323)
    if V:
        bt = np.concatenate([bt, Bm[verts].T], axis=1)
    kx = K + V + (4 - ((K + V - 1024) % 4)) % 4
    bt = np.ascontiguousarray(
        np.pad(bt, ((0, 0), (0, kx - bt.shape[1]))).astype(np.float32))
    epsr = np.ascontiguousarray(eps.transpose(1, 0, 2).reshape(B_SZ, S * LAT))
    lab32 = labels.astype(np.int32)
    if use_b:
        # softmax(l + b_n) = exp(l - hullmax) * exp(b_n - max b) / sum
        eb = np.exp(b - b.max()).astype(np.float32)

    in_maps = []
    for c in range(NCORES):
        c0, c1 = c * BS, (c + 1) * BS
        m = dict(
            x=x[c0:c1],
            epsr=epsr[c0:c1],
            labs=np.ascontiguousarray(lab32[c0:c1].reshape(NT, P).T),
            w2=w2, biasrow=biasrow, cb=cb, btab=bt, atab=A,
            ident=np.eye(P, dtype=np.float32),
        )
        if use_b:
            m["ebtab"] = eb
        in_maps.append(m)

    res = bass_utils.run_bass_kernel_spmd(nc, in_maps, core_ids=list(range(NCORES)))

    zp = np.concatenate([r["zp"] for r in res.results], axis=1)
    mp = np.concatenate([r["mp"] for r in res.results], axis=0)
    mu = np.concatenate([r["muh"] for r in res.results], axis=0)
    sd = np.concatenate([r["sdh"] for r in res.results], axis=0)
    return zp, mp, mu, sd


# revision 41
# speedup vs baseline: 1.0212x; 1.0212x over previous
"""Trainium2 Bass kernel for nn_CPProfileSurrogate (CP-factored profile surrogate).

Data-parallel over batch across 8 NeuronCores. Each core handles B/8 = 4096
rows:
    mu_h  = x @ mu_w.T + mu_b                       (PE, contraction 256)
    std_h = softplus(x @ std_w.T + std_b)           (ACT)
    h[s]  = mu_h + std_h * eps[s]                   (DVE)
    z     = h @ C ; t = A[g] * z                    (DVE, tiny contractions)
    logits= t @ B.T (+ b[g])                        (PE, contraction R=4)
    out   = softmax(logits)                         (DVE max, ACT exp, scale)

b[g] is handled via a gathered exp(b) factor only when b != 0 (the common
case b == 0 skips it): softmax(l + bg) == exp(l - m) * exp(bg) / sum.
"""

import numpy as np

import concourse.bass as bass
import concourse.tile as tile
import concourse.mybir as mybir
from concourse import bass_utils

NCORES = 8
B_SZ = 32768
S = 4
D_IN = 256
LAT = 8
NBINS = 300
K = 1323
R = 4

P = 128
BS = B_SZ // NCORES          # rows per core
NT = BS // P                 # batch tiles per core
NSLOT = S + 1                # 4 mc samples + 1 mean path
F32 = mybir.dt.float32
F32R = mybir.dt.float32r    # 4-byte fp32, 4-xbus streaming: 1 cyc/row on PE
I32 = mybir.dt.int32

# Hull-max columns appended to the logits matmul: max_k t.B_k equals the max
# over convex-hull vertices of B's rows, so the softmax max costs V extra
# matmul columns + a short reduce instead of a 1323-wide reduce. VMAX keeps
# the padded logits tile within 3 PSUM banks (1536 fp32).
VMAX = 213


def _split_multiwait(nc, max_waits=1):
    """The walrus build in this container rejects >1 sync-wait per
    instruction; hoist extra waits onto preceding single-wait NoOps."""
    for f in nc.m.functions:
        for bb in f.blocks:
            insts = bb.instructions
            new = []
            changed = False
            for inst in insts:
                si = inst.sync_info
                waits = list(si.on_wait) if si is not None else []
                if len(waits) > max_waits:
                    extra, keep = waits[:-max_waits], waits[-max_waits:]
                    for k, w in enumerate(extra):
                        new.append(mybir.InstNoOp(
                            name=f"{inst.name}_sw{k}",
                            engine=inst.engine,
                            bass_nofuse=True,
                            sync_info=mybir.SyncInfo(on_wait=[w], on_update=[]),
                        ))
                    inst.sync_info = mybir.SyncInfo(
                        on_wait=keep, on_update=list(si.on_update))
                    changed = True
                new.append(inst)
            if changed:
                bb.instructions = new


_NC_CACHE = {}


def build_nc(use_b: bool, V: int = 0):
    """V = number of appended hull-max columns (0 -> plain full reduce_max)."""
    key = (use_b, V)
    if key in _NC_CACHE:
        return _NC_CACHE[key]
    # fp32r matmuls need 4-aligned moving widths: zero-pad the table
    KX = K + V + (4 - ((K + V - 1024) % 4)) % 4
    nc = bass.Bass("TRN2", target_bir_lowering=False, debug=False)

    PKW = P + 32 + 32 + NT + NT * S * LAT  # ident|ws|cb|labs|eps columns
    x_d = nc.dram_tensor("x", [BS, D_IN], F32, kind="ExternalInput")
    pk_d = nc.dram_tensor("pack", [P, PKW], F32, kind="ExternalInput")
    bias_d = nc.dram_tensor("biasrow", [1, 16], F32, kind="ExternalInput")
    bt_d = nc.dram_tensor("btab", [R, KX], F32R, kind="ExternalInput")
    a_d = nc.dram_tensor("atab", [NBINS, R], F32, kind="ExternalInput")
    if use_b:
        eb_d = nc.dram_tensor("ebtab", [NBINS, K], F32, kind="ExternalInput")

    zp_d = nc.dram_tensor("zp", [S, BS, K], F32, kind="ExternalOutput")
    mp_d = nc.dram_tensor("mp", [BS, K], F32, kind="ExternalOutput")
    mu_d = nc.dram_tensor("muh", [BS, LAT], F32, kind="ExternalOutput")
    sd_d = nc.dram_tensor("sdh", [BS, LAT], F32, kind="ExternalOutput")

    AF = mybir.ActivationFunctionType
    AX = mybir.AxisListType
    OP = mybir.AluOpType

    with tile.TileContext(nc) as tc:
        with (
            tc.tile_pool(name="const", bufs=1) as cp,
            tc.tile_pool(name="work", bufs=4) as wp,
            tc.tile_pool(name="osb", bufs=8) as op_,
            tc.tile_pool(name="psL", bufs=2, space="PSUM") as psL,
            tc.tile_pool(name="psS", bufs=2, space="PSUM") as psS,
        ):
            # one packed DMA for [ident | ws | cb | labs | eps] — startup is
            # HWDGE/SEQ issuance-bound, so fewer, bigger loads
            pk = cp.tile([P, PKW], F32)
            nc.scalar.dma_start(pk[:], pk_d.ap()[:])
            o_ws, o_cb, o_lab, o_eps = P, P + 32, P + 64, P + 64 + NT
            ident = pk[:, 0:P]
            ws = pk[:, o_ws:o_cb].rearrange("p (c n) -> p c n", c=2)
            cb = pk[:, o_cb:o_lab]
            labs = pk[:, o_lab:o_eps].bitcast(I32)
            biasr = cp.tile([1, 16], F32)
            nc.scalar.dma_start(biasr[:], bias_d.ap()[:])
            ones1 = cp.tile([1, P], F32)
            nc.gpsimd.memset(ones1[:], 1.0)
            bt = cp.tile([R, KX], F32R)
            nc.scalar.dma_start(bt[:], bt_d.ap()[:])

            # bulk-prefetch x by groups: fills the otherwise-idle DMA startup
            # window; early groups small so tile 0 starts quickly
            GROUPS = [(0, 2), (2, 8), (8, 20), (20, NT)]
            xg, gof = [], {}
            xr = x_d.ap().rearrange("(i p) d -> p i d", p=P)
            for gi, (g0, g1) in enumerate(GROUPS):
                xt_g = cp.tile([P, g1 - g0, D_IN], F32)
                nc.scalar.dma_start(xt_g[:], xr[:, g0:g1, :])
                for i in range(g0, g1):
                    gof[i] = (gi, i - g0)
                xg.append(xt_g)

            def frontend(i):
                """Loads + linear + h/z/t + per-slot transposes for tile i.
                Returns what the softmax backend needs."""
                r0, r1 = i * P, (i + 1) * P
                gi, go = gof[i]
                xt = xg[gi][:, go, :]
                ep = pk[:, o_eps + i * S * LAT:o_eps + (i + 1) * S * LAT]
                an = wp.tile([P, R], F32, tag="an")
                nc.gpsimd.indirect_dma_start(
                    out=an[:], out_offset=None,
                    in_=a_d.ap()[:],
                    in_offset=bass.IndirectOffsetOnAxis(ap=labs[:, i:i + 1], axis=0),
                )
                ebn = None
                if use_b:
                    ebn = wp.tile([P, K], F32, tag="ebn")
                    nc.gpsimd.indirect_dma_start(
                        out=ebn[:], out_offset=None,
                        in_=eb_d.ap()[:],
                        in_offset=bass.IndirectOffsetOnAxis(ap=labs[:, i:i + 1], axis=0),
                    )

                # x.T via PE so the linear layer can contract over D_IN
                xtp = psS.tile([P, D_IN], F32, tag="pss")
                nc.tensor.transpose(xtp[:, 0:P], xt[:, 0:P], ident)
                nc.tensor.transpose(xtp[:, P:D_IN], xt[:, P:D_IN], ident)
                xts = wp.tile([P, D_IN], F32, tag="xts")
                nc.vector.tensor_copy(xts[:], xtp[:])

                # [mu_h | std_pre] = x @ [mu_w|std_w].T + [mu_b|std_b]
                lin = psS.tile([P, 16], F32, tag="pss")
                nc.tensor.matmul(out=lin[:], lhsT=xts[:, 0:P], rhs=ws[:, 0, :],
                                 start=True, stop=False)
                nc.tensor.matmul(out=lin[:], lhsT=xts[:, P:D_IN], rhs=ws[:, 1, :],
                                 start=False, stop=False)
                nc.tensor.matmul(out=lin[:], lhsT=ones1[:], rhs=biasr[:],
                                 start=False, stop=True)

                # h_all: slots 0..3 = mu+std*eps[s], slot 4 = mu (mean path)
                ha = wp.tile([P, NSLOT * LAT], F32, tag="ha")
                nc.vector.tensor_copy(ha[:, S * LAT:], lin[:, 0:LAT])
                # softplus(x) = ln(exp(x) + 1); this ACT table build has no
                # native softplus, but exp and ln share one table set
                sd = wp.tile([P, LAT], F32, tag="sd")
                nc.scalar.activation(sd[:], lin[:, LAT:16], AF.Exp)
                nc.scalar.activation(sd[:], sd[:], AF.Ln, bias=1.0)
                # small stores ride the idle Pool SWDGE queue so they can't
                # head-block the SP FIFO that carries the big zp stores
                nc.gpsimd.dma_start(mu_d.ap()[r0:r1, :], ha[:, S * LAT:])
                nc.gpsimd.dma_start(sd_d.ap()[r0:r1, :], sd[:])

                ha4 = ha[:, 0:S * LAT].rearrange("p (s l) -> p s l", s=S)
                sdb = sd[:].rearrange("p (o l) -> p o l", o=1).to_broadcast([P, S, LAT])
                mub = ha[:, S * LAT:].rearrange("p (o l) -> p o l", o=1).to_broadcast([P, S, LAT])
                ep4 = ep.rearrange("p (s l) -> p s l", s=S)
                nc.vector.tensor_tensor(out=ha4, in0=ep4, in1=sdb, op=OP.mult)
                nc.vector.tensor_tensor(out=ha4, in0=ha4, in1=mub, op=OP.add)

                # z[p, slot, r] = sum_l h[p, slot, l] * C[l, r]
                prod = wp.tile([P, NSLOT, R, LAT], F32, tag="prod")
                hab = ha[:].rearrange("p (s o l) -> p s o l", s=NSLOT, o=1) \
                           .to_broadcast([P, NSLOT, R, LAT])
                cbb = cb.rearrange("p (o r l) -> p o r l", o=1, r=R) \
                           .to_broadcast([P, NSLOT, R, LAT])
                nc.vector.tensor_tensor(out=prod[:], in0=hab, in1=cbb, op=OP.mult)
                zt = wp.tile([P, NSLOT, R], F32, tag="zt")
                nc.vector.reduce_sum(out=zt[:], in_=prod[:], axis=AX.X)
                anb = an[:].rearrange("p (o r) -> p o r", o=1).to_broadcast([P, NSLOT, R])
                nc.vector.tensor_tensor(out=zt[:], in0=zt[:], in1=anb, op=OP.mult)

                # transpose each slot's t -> (R, P) so PE can contract over R
                # (PE/PSUM partition bases must be 32-aligned, so one slot at
                # a time, each landing at partition 0)
                tts = []
                for s in range(NSLOT):
                    ttp_s = psS.tile([R, P], F32, tag="pss")
                    nc.tensor.transpose(ttp_s[:], zt[:, s, :], ident)
                    t_s = wp.tile([R, P], F32R, tag=f"tts{s}")
                    nc.vector.tensor_copy(t_s[:], ttp_s[:])
                    tts.append(t_s)
                return tts, ebn

            def backend(i, st, slots):
                """Logits + softmax + stores for tile i, given softmax slots."""
                tts, ebn = st
                r0, r1 = i * P, (i + 1) * P
                kch = [(0, 512), (512, 1024), (1024, KX)]
                for s in slots:
                    L = psL.tile([P, KX], F32, tag="L")
                    for (c0, c1) in kch:
                        nc.tensor.matmul(out=L[:, c0:c1],
                                         lhsT=tts[s][:],
                                         rhs=bt[:, c0:c1],
                                         start=True, stop=True)
                    nmax = wp.tile([P, 1], F32, tag="nmax")
                    if V:
                        # appended hull columns carry max_k(t . B_k) exactly
                        nc.vector.reduce_max(out=nmax[:], in_=L[:, K:K + V], axis=AX.X)
                    else:
                        nc.vector.reduce_max(out=nmax[:], in_=L[:, 0:K], axis=AX.X)
                    nc.vector.tensor_scalar_mul(nmax[:], nmax[:], -1.0)
                    osb = op_.tile([P, K], F32, tag="osb")
                    ssum = wp.tile([P, 1], F32, tag="ssum")
                    if use_b:
                        tmp = wp.tile([P, K], F32, tag="tmpe")
                        nc.scalar.activation(tmp[:], L[:, 0:K], AF.Exp, bias=nmax[:])
                        nc.vector.tensor_tensor(out=osb[:], in0=tmp[:], in1=ebn[:],
                                                op=OP.mult)
                        nc.vector.reduce_sum(out=ssum[:], in_=osb[:], axis=AX.X)
                    else:
                        nc.scalar.activation(osb[:], L[:, 0:K], AF.Exp, bias=nmax[:],
                                             accum_out=ssum[:])
                    rinv = wp.tile([P, 1], F32, tag="rinv")
                    nc.vector.reciprocal(rinv[:], ssum[:])
                    nc.vector.tensor_scalar_mul(osb[:], osb[:], rinv[:])
                    if s < S:
                        nc.sync.dma_start(zp_d.ap()[s, r0:r1, :], osb[:])
                    else:
                        nc.sync.dma_start(mp_d.ap()[r0:r1, :], osb[:])

            # 1-deep software pipeline: emit tile i+1's frontend before tile
            # i's softmax loop so the in-order PE always has store-feeding
            # matmul work queued behind the (short) next-tile prep.
            st = frontend(0)
            for i in range(NT):
                nxt = frontend(i + 1) if i + 1 < NT else None
                backend(i, st, range(NSLOT))
                st = nxt

    _split_multiwait(nc)
    _NC_CACHE[key] = nc
    return nc


def kernel(x, eps, A, B, C, b, mu_w, mu_b, std_w, std_b, group_labels):
    x = np.ascontiguousarray(np.asarray(x, np.float32))
    eps = np.asarray(eps, np.float32)
    A = np.ascontiguousarray(np.asarray(A, np.float32))
    Bm = np.ascontiguousarray(np.asarray(B, np.float32))
    C = np.asarray(C, np.float32)
    b = np.asarray(b, np.float32)
    mu_w = np.asarray(mu_w, np.float32)
    mu_b = np.asarray(mu_b, np.float32)
    std_w = np.asarray(std_w, np.float32)
    std_b = np.asarray(std_b, np.float32)
    labels = np.asarray(group_labels)

    use_b = bool(np.any(b))

    # convex-hull vertices of B's rows: the row-max of t @ B.T is attained at
    # a hull vertex, so V extra matmul columns give the softmax max exactly
    try:
        from scipy.spatial import ConvexHull
        verts = list(np.sort(ConvexHull(Bm.astype(np.float64)).vertices))
        # fp32r matmuls need 4-aligned moving widths; pad the last chunk
        while (K - 1024 + len(verts)) % 4:
            verts.append(verts[0])
        verts = np.asarray(verts)
        V = len(verts) if len(verts) <= VMAX else 0
    except Exception:
        verts, V = None, 0

    nc = build_nc(use_b, V)

    w2 = np.ascontiguousarray(np.concatenate([mu_w, std_w], 0).T)      # (256,16)
    biasrow = np.concatenate([mu_b, std_b])[None, :].copy()            # (1,16)
    cb = np.tile(np.ascontiguousarray(C.T).reshape(1, R * LAT), (P, 1))  # (128,32)
    bt = np.ascontiguousarray(Bm.T)                                    # (4,1323)
    if V:
        bt = np.concatenate([bt, Bm[verts].T], axis=1)
    kx = K + V + (4 - ((K + V - 1024) % 4)) % 4
    bt = np.ascontiguousarray(
        np.pad(bt, ((0, 0), (0, kx - bt.shape[1]))).astype(np.float32))
    epsr = np.ascontiguousarray(eps.transpose(1, 0, 2).reshape(B_SZ, S * LAT))
    lab32 = labels.astype(np.int32)
    if use_b:
        # softmax(l + b_n) = exp(l - hullmax) * exp(b_n - max b) / sum
        eb = np.exp(b - b.max()).astype(np.float32)

    in_maps = []
    for c in range(NCORES):
        c0, c1 = c * BS, (c + 1) * BS
        lab_t = np.ascontiguousarray(lab32[c0:c1].reshape(NT, P).T)
        eps_t = epsr[c0:c1].reshape(NT, P, S * LAT).transpose(1, 0, 2).reshape(P, -1)
        pack = np.concatenate([
            np.eye(P, dtype=np.float32),
            w2.reshape(2, P, 16).transpose(1, 0, 2).reshape(P, 32),
            cb,
            lab_t.view(np.float32),
            eps_t,
        ], axis=1).astype(np.float32, copy=False)
        m = dict(
            x=x[c0:c1],
            pack=np.ascontiguousarray(pack),
            biasrow=biasrow, btab=bt, atab=A,
        )
        if use_b:
            m["ebtab"] = eb
        in_maps.append(m)

    res = bass_utils.run_bass_kernel_spmd(nc, in_maps, core_ids=list(range(NCORES)))

    zp = np.concatenate([r["zp"] for r in res.results], axis=1)
    mp = np.concatenate([r["mp"] for r in res.results], axis=0)
    mu = np.concatenate([r["muh"] for r in res.results], axis=0)
    sd = np.concatenate([r["sdh"] for r in res.results], axis=0)
    return zp, mp, mu, sd
